# revision 1
# baseline (speedup 1.0000x reference)
"""Trainium2 Bass kernel for nn_EntropicOTQuantileRegression.

Reference computation (N=1024, M=2048, DX=48, DY=8, H=64, EPS=1e-7):
    hx = X @ W1[:DX]                                  [n, h]
    hu = U @ W1[DX:]                                  [m, h]
    h1 = softplus(hx[:,None,:] + hu[None,:,:] + b1)   [n, m, h]
    h2 = softplus(h1 @ W2 + b2)                       [n, m, h]
    phi = (h2 @ W3)[..., 0] + b3[0]                   [n, m]
    slack = Y @ U.T - phi
    psi = EPS*(logsumexp((slack - rowmax)/EPS, axis=1) - log(M)) + rowmax

Sharding: data-parallel over n. Each of the 8 cores gets 128 rows of X/Y and
replicates U + MLP weights. No collectives.

Design (583us baseline -> ~260us):
- Partition layout stacks two n-rows (h=64: 128 partitions hold rows i, i+64).
- Layer-1 pre-activation is separable: exp(hx+hu+b1) = exp(hx+b1)*exp(hu), so
  softplus1 = Ln(Ex2[:,i]*Eu2 + 1) is ONE ScalarE op (per-partition scale AP).
- All Exp/Ln ScalarE ops share the `natural_log_exp_and_others` HW table via
  the get_activation_tables patch (otherwise the compiler reloads tables 2x
  per iteration, 169us).
- The ln(1+x) work is split three ways per layer and balanced so ScalarE,
  VectorE and GPSIMD all run ~80% busy: a table-Ln head on ScalarE
  ([0, CSPLIT*)), a 2-op bf16 "Mitchell" bit-trick chain on the VectorE 4x
  path, and the same chain on the otherwise-idle Pool engine for the tail.
- phi is accumulated quad-packed in PSUM (4 iterations per [128, M] tile via
  tile_position column offsets), staged to SBUF once per quad.
- Epilogue: EPS=1e-7 collapses the f32 logsumexp to a row max exactly, so
  psi = rowmax(cost - phi) - EPS*log(M) - b3 with no Exp/Ln pass.
- Prologue: constants ride the ACT hardware DMA queue while X/Y/U stream on
  the SP queue (U in one strided DMA); big prologue matmuls are bf16 with
  doubled stationaries to dodge the 4x f32 cost at the cold-PE low pstate.
"""

import math
from contextlib import ExitStack

import numpy as np

import concourse.bass as bass
import concourse.bacc as bacc
import concourse.tile as tile
from concourse import mybir
from concourse.bass_utils import run_bass_kernel_spmd
from concourse.masks import make_identity

# Problem constants (hardcoded; kernel.py must be self-contained).
N, M = 1024, 2048
DX, DY = 48, 8
H = 64
EPS = 1e-7
N_CORES = 8
NLOC = N // N_CORES  # 128 rows per core
F32 = mybir.dt.float32
BF16 = mybir.dt.bfloat16
U16 = mybir.dt.uint16
AF = mybir.ActivationFunctionType
ALU = mybir.AluOpType

# Ln split: columns [0, CSPLIT*) stay on ScalarE (exact table Ln); the rest
# go to a 2-op VectorE "Mitchell" bit-trick: for bf16 t >= 1,
#   ln(t) ~ (bits_u16(t) - C) * ln2/128,
# since bits(t) = 128*(log2 t + 127 + eps(u)), eps in [0, 0.0861]. C centers
# eps; worst-case h-error ~0.03 which is ~15x under the psi error budget
# (numpy end-to-end: l2 rel 0.0019 with ALL columns on the chain).
# CSPLIT1 = Ln1 (layer-1) ScalarE head, CSPLIT = Ln2 (layer-2) ScalarE head;
# both engines run ~85% busy at the balance point.
CSPLIT = 448
CSPLIT1 = 640
# Pool (GPSIMD) tail widths per layer: columns [M-PW, M) run the same 2-op
# Mitchell chain on the otherwise-idle Pool engine (~0.6 impl efficiency).
POOLW = 448
POOLW1 = 384
# Columns [M-EXPW, M) also take the *Exp* via the inverse Mitchell trick on
# the DVE (bits = round(x*128/ln2 + C) bitcast to bf16 ~ e^x), shrinking the
# ScalarE Exp2 pass. Emitted BEFORE Exp2 so it reads pre2 with zero wait.
EXPW = 0
LN2 = math.log(2.0)
LOG2E128 = 128.0 / LN2
MITCH_C = 16256 - 6  # 127<<7 minus eps-centering

_CACHE = {}


def _patch_act_tables():
    """Make Exp/Ln resolve uniquely to the combined natural_log_exp_and_others
    table so `insert_act_table_loads` hoists ONE load instead of thrashing.

    The greedy table chooser picks the first set containing each function:
    Exp -> exp_and_others, Ln -> natural_log, so the baseline's Ln/Exp/Ln
    per-iteration sequence reloaded the ACT table 2x per iteration (132 loads,
    169us, 29% of kernel time). Filtering Exp/Ln out of every OTHER set (order
    and set count preserved, so act_func_set_id indices stay valid) forces the
    combined table, which genuinely contains both functions on gen3 HW.
    """
    if getattr(bacc, "_act_tables_patched", False):
        return
    orig = bacc.get_activation_tables
    AFT = mybir.ActivationFunctionType

    def patched(arch):
        tabs = dict(orig(arch))
        combined = "natural_log_exp_and_others"
        if combined in tabs and {AFT.Exp, AFT.Ln} <= tabs[combined]:
            tabs = {
                name: (s if name == combined else s - {AFT.Exp, AFT.Ln})
                for name, s in tabs.items()
            }
        return tabs

    bacc.get_activation_tables = patched
    bacc._act_tables_patched = True


def build_program(repeats=1, csplit=None, loop_n=0, csplit1=None, poolw=None,
                  poolw1=None, expw=None):
    global CSPLIT, CSPLIT1, POOLW, POOLW1, EXPW
    if csplit is not None:
        CSPLIT = csplit
    if csplit1 is not None:
        CSPLIT1 = csplit1
    if poolw is not None:
        POOLW = poolw
    if poolw1 is not None:
        POOLW1 = poolw1
    if expw is not None:
        EXPW = expw
    _patch_act_tables()
    nc = bacc.Bacc(
        "TRN2",
        target_bir_lowering=False,
        debug=False,
        enable_asserts=False,
        num_devices=N_CORES,
    )

    X = nc.dram_tensor("X", (NLOC, DX), F32, kind="ExternalInput").ap()
    U = nc.dram_tensor("U", (M, DY), F32, kind="ExternalInput").ap()
    Y = nc.dram_tensor("Y", (NLOC, DY), F32, kind="ExternalInput").ap()
    W1 = nc.dram_tensor("W1", (DX + DY, H), F32, kind="ExternalInput").ap()
    b1 = nc.dram_tensor("b1", (H,), F32, kind="ExternalInput").ap()
    W2 = nc.dram_tensor("W2", (H, H), F32, kind="ExternalInput").ap()
    b2 = nc.dram_tensor("b2", (H,), F32, kind="ExternalInput").ap()
    W3 = nc.dram_tensor("W3", (H, 1), F32, kind="ExternalInput").ap()
    b3 = nc.dram_tensor("b3", (1,), F32, kind="ExternalInput").ap()
    out = nc.dram_tensor("out", (NLOC, 1), F32, kind="ExternalOutput").ap()

    with tile.TileContext(nc) as tc:
        if loop_n:
            with tc.For_i(0, loop_n, 1):
                with ExitStack() as ctx:
                    _body(ctx, tc, nc, X, U, Y, W1, b1, W2, b2, W3, b3, out)
        else:
            for _ in range(repeats):
                with ExitStack() as ctx:
                    _body(ctx, tc, nc, X, U, Y, W1, b1, W2, b2, W3, b3, out)

    nc.compile()
    return nc


def _body(ctx, tc, nc, X, U, Y, W1, b1, W2, b2, W3, b3, out):
    NITER = NLOC // 2  # 64: each iteration handles rows (i, i+64)

    const = ctx.enter_context(tc.tile_pool(name="const", bufs=1))
    big = ctx.enter_context(tc.tile_pool(name="big", bufs=1))

    # --- small SBUF constants -------------------------------------------
    # Each dma_start costs ~625ns of queue time, so constants are fused into
    # single DMAs (stride-0 repeat APs for the x2 partition stacking) and
    # routed via the ScalarE hardware DMA queue so X/Y/U can stream on the SP
    # queue in parallel. (gpsimd dma_start is SWDGE: ~1us serial on Pool.)
    ident = const.tile([128, 128], F32)
    make_identity(nc, ident)

    rep2 = lambda t, inner: bass.AP(tensor=t, offset=0, ap=[[0, 2]] + inner)

    W1a = const.tile([DX, H], F32)
    nc.scalar.dma_start(out=W1a, in_=W1[0:DX, :])
    W1b = const.tile([DY, H], F32)
    nc.scalar.dma_start(out=W1b, in_=W1[DX : DX + DY, :])

    # b1/b2 stacked twice on 128 partitions: partition p holds b[p % 64]
    b1s = const.tile([128, 1], F32)
    nc.scalar.dma_start(out=b1s, in_=rep2(b1.tensor, [[1, H]]))
    b2s = const.tile([128, 1], F32)
    nc.scalar.dma_start(out=b2s, in_=rep2(b2.tensor, [[1, H]]))
    b3s = const.tile([128, 1], F32)
    nc.scalar.dma_start(out=b3s, in_=b3.unsqueeze(1).partition_broadcast(128))

    # W2 block-diagonal stack [128,128] bf16: [[W2, 0], [0, W2]]
    W2f = const.tile([128, H], F32)
    nc.scalar.dma_start(out=W2f, in_=rep2(W2.tensor, [[H, H], [1, H]]))
    W2s = const.tile([128, 128], BF16)
    nc.vector.memset(W2s, 0.0)
    nc.vector.tensor_copy(W2s[0:H, 0:H], W2f[0:H, :])
    nc.vector.tensor_copy(W2s[H : 2 * H, H : 2 * H], W2f[H : 2 * H, :])

    # W3 stack [128, 32] bf16: cols 0/1 = the two W3 halves, cols 2..31 zero
    # (32-wide so each quad phi matmul writes a full 32-partition col group).
    W3f = const.tile([128, 1], F32)
    nc.scalar.dma_start(out=W3f, in_=rep2(W3.tensor, [[1, H]]))
    W3s = const.tile([128, 32], BF16)
    nc.vector.memset(W3s, 0.0)
    nc.vector.tensor_copy(W3s[0:H, 0:1], W3f[0:H, :])
    nc.vector.tensor_copy(W3s[H : 2 * H, 1:2], W3f[H : 2 * H, :])

    # bf16 doubled stationaries so the big prologue matmuls run 1 col/cycle
    # (f32 matmuls cost 4x cycles and hit the cold-PE low pstate: ~2.4us per
    # 512-col chunk at kernel start).
    W1ab = const.tile([DX, H], BF16)
    nc.vector.tensor_copy(W1ab, W1a)
    W1bb2 = const.tile([DY, 128], BF16)  # [W1b | W1b]
    nc.vector.tensor_copy(W1bb2[:, 0:H], W1b)
    nc.vector.tensor_copy(W1bb2[:, H : 2 * H], W1b)

    # --- transposes (PE) -------------------------------------------------
    X_T = const.tile([DX, 128], BF16)  # X^T
    Y_T = const.tile([DY, 128], BF16)  # Y^T
    U_T = const.tile([DY, M], BF16)  # U^T
    with tc.tile_pool(name="psumA", bufs=1, space="PSUM") as psA, tc.tile_pool(
        name="ld", bufs=4
    ) as ld:
        X_sb = ld.tile([128, DX], F32, tag="xy")
        nc.sync.dma_start(out=X_sb, in_=X)
        X_T_ps = psA.tile([DX, 128], F32, tag="xyt")
        nc.tensor.transpose(X_T_ps, X_sb, ident)
        nc.vector.tensor_copy(X_T, X_T_ps)

        # Y rows loaded in interleaved order q = 2i+p <-> n = i + 64p, so that
        # cost rows line up with the phi layout written by the main loop.
        Y_sb = ld.tile([128, DY], F32, tag="xy")
        Y_perm = bass.AP(
            tensor=Y.tensor,
            offset=Y.offset,
            ap=[[DY, NITER], [NITER * DY, 2], [1, DY]],
        )
        nc.sync.dma_start(out=Y_sb, in_=Y_perm)
        Y_T_ps = psA.tile([DY, 128], F32, tag="xyt")
        nc.tensor.transpose(Y_T_ps, Y_sb, ident)
        nc.vector.tensor_copy(Y_T, Y_T_ps)

        # U in ONE DMA: partition p, column group k holds U[k*128+p, :]
        U_all = ld.tile([128, (M // 128) * DY], F32, tag="uall")
        U_perm = bass.AP(
            tensor=U.tensor,
            offset=0,
            ap=[[DY, 128], [128 * DY, M // 128], [1, DY]],
        )
        nc.sync.dma_start(out=U_all, in_=U_perm)
        U_T_ps = psA.tile([DY, M], F32)
        for k in range(M // 128):
            nc.tensor.transpose(
                U_T_ps[:, k * 128 : (k + 1) * 128],
                U_all[:, k * DY : (k + 1) * DY],
                ident,
            )
        nc.vector.tensor_copy(U_T, U_T_ps)

    # --- Ex2 = exp(hx+b1) stacked, Eu2 = exp(hu) stacked, cost ----------
    # Eu2 in bf16: read by both the ScalarE Ln1 head and the DVE chain (2-byte
    # operands keep the chain in DVE 2x perf mode).
    Ex2 = const.tile([128, NITER], F32)
    Eu2 = big.tile([128, M], BF16)
    cost = big.tile([128, M], F32)
    with tc.tile_pool(name="psumB", bufs=1, space="PSUM") as psB, tc.tile_pool(
        name="psumC", bufs=2, space="PSUM"
    ) as psC:
        # hx2[p, i] = (X @ W1a)[i + 64*(p>=64), p%64]; stacked columns.
        hx2_ps = psB.tile([128, NITER], F32)
        nc.tensor.matmul(hx2_ps[0:H, :], W1ab, X_T[:, 0:NITER], start=True, stop=True)
        nc.tensor.matmul(
            hx2_ps[H : 2 * H, :],
            W1ab,
            X_T[:, NITER : 2 * NITER],
            start=True,
            stop=True,
            tile_position=(0, 64),
        )
        nc.scalar.activation(Ex2, hx2_ps, AF.Exp, bias=b1s, scale=1.0)

        # hu2 = U @ W1b replicated on both partition halves via the doubled
        # stationary [W1b | W1b] (one matmul per chunk instead of two).
        hu2_ps = psB.tile([128, M], F32)
        for j in range(M // 512):
            sl = slice(j * 512, (j + 1) * 512)
            nc.tensor.matmul(hu2_ps[:, sl], W1bb2, U_T[:, sl], start=True, stop=True)
        nc.scalar.activation(Eu2, hu2_ps, AF.Exp, bias=0.0, scale=1.0)

        # cost = Y @ U.T  -> [128, 2048] f32
        for j in range(M // 512):
            sl = slice(j * 512, (j + 1) * 512)
            cost_ps = psC.tile([128, 512], F32, tag="cost")
            nc.tensor.matmul(cost_ps, Y_T, U_T[:, sl], start=True, stop=True)
            nc.vector.tensor_copy(cost[:, sl], cost_ps)

    # Kb2 = b2*128/ln2 + C: per-partition constant for the inverse-Mitchell
    # exp tail (bits = pre2*128/ln2 + Kb2).
    Kb2 = const.tile([128, 1], F32)
    nc.vector.tensor_scalar(
        out=Kb2, in0=b2s, scalar1=LOG2E128, scalar2=float(MITCH_C),
        op0=ALU.mult, op1=ALU.add,
    )

    # --- main loop -------------------------------------------------------
    # phi rows live in interleaved order: partition q holds row n(q)=q//2+64*(q%2)
    phi = big.tile([128, M], F32)
    h1_pool = ctx.enter_context(tc.tile_pool(name="h1", bufs=4))
    e2_pool = ctx.enter_context(tc.tile_pool(name="e2", bufs=4))
    h2_pool = ctx.enter_context(tc.tile_pool(name="h2", bufs=4))
    chain_pool = ctx.enter_context(tc.tile_pool(name="chain", bufs=2))
    stage_pool = ctx.enter_context(tc.tile_pool(name="stage", bufs=2))
    pre2_pool = ctx.enter_context(tc.tile_pool(name="pre2", bufs=1, space="PSUM"))
    phi_pool = ctx.enter_context(tc.tile_pool(name="phip", bufs=1, space="PSUM"))

    # Column layout per layer: [0, CS) ScalarE table-Ln | [CS, M-PW) DVE
    # Mitchell | [M-PW, M) Pool Mitchell.
    C2W = M - CSPLIT - POOLW  # layer-2 DVE-chain column count
    dsl = slice(CSPLIT, M - POOLW)
    psl = slice(M - POOLW, M)
    C1W = M - CSPLIT1 - POOLW1  # layer-1 DVE-chain column count
    dsl1 = slice(CSPLIT1, M - POOLW1)
    psl1 = slice(M - POOLW1, M)

    def emit_mitchell(eng, out_ap, t_bf16):
        # ln(t) ~ (bits_u16(t) - C) * ln2/128 for bf16 t >= 1 (one fused
        # tensor_scalar: int subtract, then multiply converts to float out).
        eng.tensor_scalar(
            out=out_ap, in0=t_bf16.bitcast(U16), scalar1=MITCH_C,
            scalar2=LN2 / 128.0, op0=ALU.subtract, op1=ALU.mult,
        )

    def emit_chain1(eng, out_ap, src_ap, W, i, tag):
        # layer-1 softplus tail: ln(Ex2[:,i]*src + 1) via Mitchell
        t1 = chain_pool.tile([128, W], BF16, tag=tag)
        eng.tensor_scalar(
            out=t1, in0=src_ap, scalar1=Ex2[:, i : i + 1],
            scalar2=1.0, op0=ALU.mult, op1=ALU.add,
        )
        emit_mitchell(eng, out_ap, t1)

    def emit_chain2(eng, out_ap, src_ap, W, tag):
        # layer-2 softplus tail: ln(src + 1) via Mitchell
        t2 = chain_pool.tile([128, W], BF16, tag=tag)
        eng.tensor_scalar_add(t2, src_ap, 1.0)
        emit_mitchell(eng, out_ap, t2)

    def emit_ln1(i):
        # softplus1: h1 = Ln(Ex2[:,i] * Eu2 + 1)   (rows i and i+64)
        h1 = h1_pool.tile([128, M], BF16, tag="h1", name=f"h1_{i}")
        if CSPLIT1:
            nc.scalar.activation(
                h1[:, 0:CSPLIT1], Eu2[:, 0:CSPLIT1], AF.Ln, bias=1.0,
                scale=Ex2[:, i : i + 1],
            )
        if C1W:
            emit_chain1(nc.vector, h1[:, dsl1], Eu2[:, dsl1], C1W, i, "t1d")
        if POOLW1:
            emit_chain1(nc.gpsimd, h1[:, psl1], Eu2[:, psl1], POOLW1, i, "t1p")
        return h1

    esl = slice(M - EXPW, M)
    assert EXPW == 0 or EXPW == POOLW, "exp tail must equal the Pool range"

    def emit_mm1(i, h1):
        # pre2 = W2s.T @ h1  (block-diag -> both halves independently)
        pre2_ps = pre2_pool.tile([128, M], F32, tag="pre2", name=f"pre2_{i}")
        for j in range(M // 512):
            sl = slice(j * 512, (j + 1) * 512)
            nc.tensor.matmul(pre2_ps[:, sl], W2s, h1[:, sl], start=True, stop=True)
        e2 = e2_pool.tile([128, M - EXPW], BF16, tag="e2", name=f"e2_{i}")
        et = (
            e2_pool.tile([128, EXPW], BF16, tag="et", name=f"et_{i}")
            if EXPW
            else None
        )
        return pre2_ps, e2, et

    def emit_exp_tail(pre2_ps, e2, et):
        # inverse-Mitchell exp for the tail columns into its OWN tile (a
        # second writer on e2 would add semaphore waits to every downstream
        # ScalarE op). Emitted AFTER the other DVE work of the body so it
        # never head-of-line-blocks the chains while waiting on mm1.
        if EXPW:
            nc.vector.tensor_scalar(
                out=et.bitcast(U16), in0=pre2_ps[:, esl],
                scalar1=LOG2E128, scalar2=Kb2, op0=ALU.mult, op1=ALU.add,
            )

    # Software pipeline: keep ScalarE's FIFO fed — emit Ln1 two iterations
    # ahead and mm1 one iteration ahead, so PE work overlaps the Ln1/chain
    # window instead of stalling the next Exp.
    h1_ahead = {0: emit_ln1(0), 1: emit_ln1(1)}
    pre2_ahead = {0: emit_mm1(0, h1_ahead.pop(0))}
    emit_exp_tail(*pre2_ahead[0])
    e2_live = {}

    def emit_h2_phi(k, phi_quad):
        # finish softplus2 for iteration k from its (already long-done) e2,
        # then its phi matmul quad piece. Runs one iteration behind Exp2 so
        # the DVE/Pool chains never wait on a just-issued ScalarE op.
        e2, et = e2_live.pop(k)
        h2 = h2_pool.tile([128, M], BF16, tag="h2")
        if CSPLIT:
            nc.scalar.activation(
                h2[:, 0:CSPLIT], e2[:, 0:CSPLIT], AF.Ln, bias=1.0, scale=1.0
            )
        if C2W:
            emit_chain2(nc.vector, h2[:, dsl], e2[:, dsl], C2W, "t2d")
        if POOLW:
            psrc = et if EXPW else e2[:, psl]
            emit_chain2(nc.gpsimd, h2[:, psl], psrc, POOLW, "t2p")
        q = k % 4
        for j in range(M // 512):
            sl = slice(j * 512, (j + 1) * 512)
            nc.tensor.matmul(
                phi_quad[32 * q : 32 * q + 32, sl], W3s, h2[:, sl],
                start=True, stop=True,
                tile_position=(0, 32 * q) if q else None,
            )
        if q == 3:
            # PSUM is not a legal DMA source: stage through SBUF on the DVE.
            phi_stage = stage_pool.tile([128, M], F32, tag="stage")
            nc.vector.tensor_copy(phi_stage, phi_quad)
            g = k // 4
            for qq in range(4):
                r = 2 * (4 * g + qq)
                nc.sync.dma_start(
                    out=phi[r : r + 2, :],
                    in_=phi_stage[32 * qq : 32 * qq + 2, :],
                )

    # phi quad tiles are consumed one iteration late, so allocate per quad
    # of the DELAYED index k = i-1.
    phi_quads = {}
    for i in range(NITER):
        pre2_ps, e2, et = pre2_ahead.pop(i)

        # softplus2 part 1: e2 = Exp(pre2 + b2) on ScalarE, table-Exp on
        # [0, M-EXPW) (the tail was already computed by the DVE into et).
        nc.scalar.activation(e2, pre2_ps[:, 0 : M - EXPW], AF.Exp,
                             bias=b2s, scale=1.0)
        e2_live[i] = (e2, et)

        # hoist next iteration's mm1 so PE runs it during the Ln1/chain window
        if i + 1 < NITER:
            pre2_ahead[i + 1] = emit_mm1(i + 1, h1_ahead.pop(i + 1))

        # softplus2 part 2 + phi for the PREVIOUS iteration
        k = i - 1
        if k >= 0:
            if k % 4 == 0:
                phi_quads[k // 4] = phi_pool.tile([128, M], F32, tag="phi", name=f"phiq_{k}")
            emit_h2_phi(k, phi_quads[k // 4])

        # exp tail for iteration i+1 (behind the chains on the DVE queue)
        if i + 1 < NITER:
            emit_exp_tail(*pre2_ahead[i + 1])

        if i + 2 < NITER:
            h1_ahead[i + 2] = emit_ln1(i + 2)

    # drain the last delayed iteration
    k = NITER - 1
    if k % 4 == 0:
        phi_quads[k // 4] = phi_pool.tile([128, M], F32, tag="phi", name=f"phiq_{k}")
    emit_h2_phi(k, phi_quads[k // 4])

    # --- final: psi = rowmax(cost - phi) - EPS*log(M) - b3 ---------------
    # With EPS=1e-7 the f32 logsumexp collapses to the row max: the exp of
    # the second-best gap underflows, so the correction is exactly -EPS*log(M)
    # (bounded by EPS*log(M) ~ 7.6e-7 in all cases — far below tolerance).
    fin = ctx.enter_context(tc.tile_pool(name="fin", bufs=1))
    # NOTE: vector.tensor_tensor_reduce wedges the device (NRT_EXEC_UNIT_
    # UNRECOVERABLE) on this stack — use separate sub + reduce_max.
    slack = big.tile([128, M], F32)
    rowmax = fin.tile([128, 1], F32)
    nc.vector.tensor_sub(slack, cost, phi)
    nc.vector.reduce_max(out=rowmax, in_=slack, axis=mybir.AxisListType.X)
    base = fin.tile([128, 1], F32)
    # base = -b3 - EPS*log(M)
    nc.vector.tensor_scalar(
        out=base, in0=b3s, scalar1=-1.0, scalar2=-EPS * math.log(M),
        op0=ALU.mult, op1=ALU.add,
    )
    psi = fin.tile([128, 1], F32)
    nc.vector.tensor_add(psi, rowmax, base)
    # psi partition q holds row n(q)=q//2+64*(q%2); un-permute via the DRAM AP.
    out_perm = bass.AP(tensor=out.tensor, offset=out.offset, ap=[[1, NITER], [NITER, 2]])
    nc.sync.dma_start(out=out_perm, in_=psi)


def kernel(**inputs):
    if "nc" not in _CACHE:
        _CACHE["nc"] = build_program()
    nc = _CACHE["nc"]

    f32 = lambda a: np.ascontiguousarray(np.asarray(a, dtype=np.float32))
    X = f32(inputs["X"])
    U = f32(inputs["U"])
    Y = f32(inputs["Y"])
    shared = dict(
        U=U,
        W1=f32(inputs["W1"]),
        b1=f32(inputs["b1"]),
        W2=f32(inputs["W2"]),
        b2=f32(inputs["b2"]),
        W3=f32(inputs["W3"]),
        b3=f32(inputs["b3"]),
    )
    in_maps = [
        dict(
            X=X[c * NLOC : (c + 1) * NLOC],
            Y=Y[c * NLOC : (c + 1) * NLOC],
            **shared,
        )
        for c in range(N_CORES)
    ]
    res = run_bass_kernel_spmd(nc, in_maps, core_ids=list(range(N_CORES)))
    return np.concatenate([res.results[c]["out"] for c in range(N_CORES)], axis=0)


if __name__ == "__main__":
    rng = np.random.default_rng(0)
    ins = {
        "X": rng.standard_normal((N, DX), dtype=np.float32),
        "U": rng.standard_normal((M, DY), dtype=np.float32),
        "Y": rng.standard_normal((N, DY), dtype=np.float32),
        "W1": (rng.standard_normal((DX + DY, H)) * 0.1).astype(np.float32),
        "b1": np.zeros(H, np.float32),
        "W2": (rng.standard_normal((H, H)) * 0.1).astype(np.float32),
        "b2": np.zeros(H, np.float32),
        "W3": (rng.standard_normal((H, 1)) * 0.1).astype(np.float32),
        "b3": np.zeros(1, np.float32),
    }
    out = kernel(**ins)
    print(out.shape, out[:4, 0])



# revision 2
# speedup vs baseline: 3.5227x; 3.5227x over previous
"""Trainium2 Bass kernel for nn_EntropicOTQuantileRegression.

Reference computation (N=1024, M=2048, DX=48, DY=8, H=64, EPS=1e-7):
    hx = X @ W1[:DX]                                  [n, h]
    hu = U @ W1[DX:]                                  [m, h]
    h1 = softplus(hx[:,None,:] + hu[None,:,:] + b1)   [n, m, h]
    h2 = softplus(h1 @ W2 + b2)                       [n, m, h]
    phi = (h2 @ W3)[..., 0] + b3[0]                   [n, m]
    slack = Y @ U.T - phi
    psi = EPS*(logsumexp((slack - rowmax)/EPS, axis=1) - log(M)) + rowmax

Sharding: data-parallel over n. Each of the 8 cores gets 128 rows of X/Y and
replicates U + MLP weights. No collectives.

Design (583us baseline -> ~260us):
- Partition layout stacks two n-rows (h=64: 128 partitions hold rows i, i+64).
- Layer-1 pre-activation is separable: exp(hx+hu+b1) = exp(hx+b1)*exp(hu), so
  softplus1 = Ln(Ex2[:,i]*Eu2 + 1) is ONE ScalarE op (per-partition scale AP).
- All Exp/Ln ScalarE ops share the `natural_log_exp_and_others` HW table via
  the get_activation_tables patch (otherwise the compiler reloads tables 2x
  per iteration, 169us).
- The ln(1+x) work is split three ways per layer and balanced so ScalarE,
  VectorE and GPSIMD all run ~80% busy: a table-Ln head on ScalarE
  ([0, CSPLIT*)), a 2-op bf16 "Mitchell" bit-trick chain on the VectorE 4x
  path, and the same chain on the otherwise-idle Pool engine for the tail.
- phi is accumulated quad-packed in PSUM (4 iterations per [128, M] tile via
  tile_position column offsets), staged to SBUF once per quad.
- Epilogue: EPS=1e-7 collapses the f32 logsumexp to a row max exactly, so
  psi = rowmax(cost - phi) - EPS*log(M) - b3 with no Exp/Ln pass.
- Prologue: constants ride the ACT hardware DMA queue while X/Y/U stream on
  the SP queue (U in one strided DMA); big prologue matmuls are bf16 with
  doubled stationaries to dodge the 4x f32 cost at the cold-PE low pstate.
"""

import math
from contextlib import ExitStack

import numpy as np

import concourse.bass as bass
import concourse.bacc as bacc
import concourse.tile as tile
from concourse import mybir
from concourse.bass_utils import run_bass_kernel_spmd
from concourse.masks import make_identity

# Problem constants (hardcoded; kernel.py must be self-contained).
N, M = 1024, 2048
DX, DY = 48, 8
H = 64
EPS = 1e-7
N_CORES = 8
NLOC = N // N_CORES  # 128 rows per core
F32 = mybir.dt.float32
BF16 = mybir.dt.bfloat16
U16 = mybir.dt.uint16
AF = mybir.ActivationFunctionType
ALU = mybir.AluOpType

# Ln split: columns [0, CSPLIT*) stay on ScalarE (exact table Ln); the rest
# go to a 2-op VectorE "Mitchell" bit-trick: for bf16 t >= 1,
#   ln(t) ~ (bits_u16(t) - C) * ln2/128,
# since bits(t) = 128*(log2 t + 127 + eps(u)), eps in [0, 0.0861]. C centers
# eps; worst-case h-error ~0.03 which is ~15x under the psi error budget
# (numpy end-to-end: l2 rel 0.0019 with ALL columns on the chain).
# CSPLIT1 = Ln1 (layer-1) ScalarE head, CSPLIT = Ln2 (layer-2) ScalarE head;
# both engines run ~85% busy at the balance point.
CSPLIT = 448
CSPLIT1 = 640
# Pool (GPSIMD) tail widths per layer. MEASURED ON HW: putting these chains
# on Pool costs ~1.1ms of stall (dependent-op wake-up latency) — the 1.34ms
# baseline collapses to ~250us with the Pool engine out of the inner loop.
POOLW = 0
POOLW1 = 0
# Columns [M-EXPW, M) also take the *Exp* via the inverse Mitchell trick on
# the DVE (bits = round(x*128/ln2 + C) bitcast to bf16 ~ e^x), shrinking the
# ScalarE Exp2 pass. Emitted BEFORE Exp2 so it reads pre2 with zero wait.
EXPW = 0
LN2 = math.log(2.0)
LOG2E128 = 128.0 / LN2
MITCH_C = 16256 - 6  # 127<<7 minus eps-centering

_CACHE = {}


def _patch_act_tables():
    """Make Exp/Ln resolve uniquely to the combined natural_log_exp_and_others
    table so `insert_act_table_loads` hoists ONE load instead of thrashing.

    The greedy table chooser picks the first set containing each function:
    Exp -> exp_and_others, Ln -> natural_log, so the baseline's Ln/Exp/Ln
    per-iteration sequence reloaded the ACT table 2x per iteration (132 loads,
    169us, 29% of kernel time). Filtering Exp/Ln out of every OTHER set (order
    and set count preserved, so act_func_set_id indices stay valid) forces the
    combined table, which genuinely contains both functions on gen3 HW.
    """
    if getattr(bacc, "_act_tables_patched", False):
        return
    orig = bacc.get_activation_tables
    AFT = mybir.ActivationFunctionType

    def patched(arch):
        tabs = dict(orig(arch))
        combined = "natural_log_exp_and_others"
        if combined in tabs and {AFT.Exp, AFT.Ln} <= tabs[combined]:
            tabs = {
                name: (s if name == combined else s - {AFT.Exp, AFT.Ln})
                for name, s in tabs.items()
            }
        return tabs

    bacc.get_activation_tables = patched
    bacc._act_tables_patched = True


def build_program(repeats=1, csplit=None, loop_n=0, csplit1=None, poolw=None,
                  poolw1=None, expw=None):
    global CSPLIT, CSPLIT1, POOLW, POOLW1, EXPW
    if csplit is not None:
        CSPLIT = csplit
    if csplit1 is not None:
        CSPLIT1 = csplit1
    if poolw is not None:
        POOLW = poolw
    if poolw1 is not None:
        POOLW1 = poolw1
    if expw is not None:
        EXPW = expw
    _patch_act_tables()
    nc = bacc.Bacc(
        "TRN2",
        target_bir_lowering=False,
        debug=False,
        enable_asserts=False,
        num_devices=N_CORES,
    )

    X = nc.dram_tensor("X", (NLOC, DX), F32, kind="ExternalInput").ap()
    U = nc.dram_tensor("U", (M, DY), F32, kind="ExternalInput").ap()
    Y = nc.dram_tensor("Y", (NLOC, DY), F32, kind="ExternalInput").ap()
    W1 = nc.dram_tensor("W1", (DX + DY, H), F32, kind="ExternalInput").ap()
    b1 = nc.dram_tensor("b1", (H,), F32, kind="ExternalInput").ap()
    W2 = nc.dram_tensor("W2", (H, H), F32, kind="ExternalInput").ap()
    b2 = nc.dram_tensor("b2", (H,), F32, kind="ExternalInput").ap()
    W3 = nc.dram_tensor("W3", (H, 1), F32, kind="ExternalInput").ap()
    b3 = nc.dram_tensor("b3", (1,), F32, kind="ExternalInput").ap()
    out = nc.dram_tensor("out", (NLOC, 1), F32, kind="ExternalOutput").ap()

    with tile.TileContext(nc) as tc:
        if loop_n:
            with tc.For_i(0, loop_n, 1):
                with ExitStack() as ctx:
                    _body(ctx, tc, nc, X, U, Y, W1, b1, W2, b2, W3, b3, out)
        else:
            for _ in range(repeats):
                with ExitStack() as ctx:
                    _body(ctx, tc, nc, X, U, Y, W1, b1, W2, b2, W3, b3, out)

    nc.compile()
    return nc


def _body(ctx, tc, nc, X, U, Y, W1, b1, W2, b2, W3, b3, out):
    NITER = NLOC // 2  # 64: each iteration handles rows (i, i+64)

    const = ctx.enter_context(tc.tile_pool(name="const", bufs=1))
    big = ctx.enter_context(tc.tile_pool(name="big", bufs=1))

    # --- small SBUF constants -------------------------------------------
    # Each dma_start costs ~625ns of queue time, so constants are fused into
    # single DMAs (stride-0 repeat APs for the x2 partition stacking) and
    # routed via the ScalarE hardware DMA queue so X/Y/U can stream on the SP
    # queue in parallel. (gpsimd dma_start is SWDGE: ~1us serial on Pool.)
    ident = const.tile([128, 128], F32)
    make_identity(nc, ident)

    rep2 = lambda t, inner: bass.AP(tensor=t, offset=0, ap=[[0, 2]] + inner)

    W1a = const.tile([DX, H], F32)
    nc.scalar.dma_start(out=W1a, in_=W1[0:DX, :])
    W1b = const.tile([DY, H], F32)
    nc.scalar.dma_start(out=W1b, in_=W1[DX : DX + DY, :])

    # b1/b2 stacked twice on 128 partitions: partition p holds b[p % 64]
    b1s = const.tile([128, 1], F32)
    nc.scalar.dma_start(out=b1s, in_=rep2(b1.tensor, [[1, H]]))
    b2s = const.tile([128, 1], F32)
    nc.scalar.dma_start(out=b2s, in_=rep2(b2.tensor, [[1, H]]))
    b3s = const.tile([128, 1], F32)
    nc.scalar.dma_start(out=b3s, in_=b3.unsqueeze(1).partition_broadcast(128))

    # W2 block-diagonal stack [128,128] bf16: [[W2, 0], [0, W2]]
    W2f = const.tile([128, H], F32)
    nc.scalar.dma_start(out=W2f, in_=rep2(W2.tensor, [[H, H], [1, H]]))
    W2s = const.tile([128, 128], BF16)
    nc.vector.memset(W2s, 0.0)
    nc.vector.tensor_copy(W2s[0:H, 0:H], W2f[0:H, :])
    nc.vector.tensor_copy(W2s[H : 2 * H, H : 2 * H], W2f[H : 2 * H, :])

    # W3 stack [128, 32] bf16: cols 0/1 = the two W3 halves, cols 2..31 zero
    # (32-wide so each quad phi matmul writes a full 32-partition col group).
    W3f = const.tile([128, 1], F32)
    nc.scalar.dma_start(out=W3f, in_=rep2(W3.tensor, [[1, H]]))
    W3s = const.tile([128, 32], BF16)
    nc.vector.memset(W3s, 0.0)
    nc.vector.tensor_copy(W3s[0:H, 0:1], W3f[0:H, :])
    nc.vector.tensor_copy(W3s[H : 2 * H, 1:2], W3f[H : 2 * H, :])

    # bf16 doubled stationaries so the big prologue matmuls run 1 col/cycle
    # (f32 matmuls cost 4x cycles and hit the cold-PE low pstate: ~2.4us per
    # 512-col chunk at kernel start).
    W1ab = const.tile([DX, H], BF16)
    nc.vector.tensor_copy(W1ab, W1a)
    W1bb2 = const.tile([DY, 128], BF16)  # [W1b | W1b]
    nc.vector.tensor_copy(W1bb2[:, 0:H], W1b)
    nc.vector.tensor_copy(W1bb2[:, H : 2 * H], W1b)

    # --- transposes (PE) -------------------------------------------------
    X_T = const.tile([DX, 128], BF16)  # X^T
    Y_T = const.tile([DY, 128], BF16)  # Y^T
    U_T = const.tile([DY, M], BF16)  # U^T
    with tc.tile_pool(name="psumA", bufs=1, space="PSUM") as psA, tc.tile_pool(
        name="ld", bufs=4
    ) as ld:
        X_sb = ld.tile([128, DX], F32, tag="xy")
        nc.sync.dma_start(out=X_sb, in_=X)
        X_T_ps = psA.tile([DX, 128], F32, tag="xyt")
        nc.tensor.transpose(X_T_ps, X_sb, ident)
        nc.vector.tensor_copy(X_T, X_T_ps)

        # Y rows loaded in interleaved order q = 2i+p <-> n = i + 64p, so that
        # cost rows line up with the phi layout written by the main loop.
        Y_sb = ld.tile([128, DY], F32, tag="xy")
        Y_perm = bass.AP(
            tensor=Y.tensor,
            offset=Y.offset,
            ap=[[DY, NITER], [NITER * DY, 2], [1, DY]],
        )
        nc.sync.dma_start(out=Y_sb, in_=Y_perm)
        Y_T_ps = psA.tile([DY, 128], F32, tag="xyt")
        nc.tensor.transpose(Y_T_ps, Y_sb, ident)
        nc.vector.tensor_copy(Y_T, Y_T_ps)

        # U in ONE DMA: partition p, column group k holds U[k*128+p, :]
        U_all = ld.tile([128, (M // 128) * DY], F32, tag="uall")
        U_perm = bass.AP(
            tensor=U.tensor,
            offset=0,
            ap=[[DY, 128], [128 * DY, M // 128], [1, DY]],
        )
        nc.sync.dma_start(out=U_all, in_=U_perm)
        U_T_ps = psA.tile([DY, M], F32)
        for k in range(M // 128):
            nc.tensor.transpose(
                U_T_ps[:, k * 128 : (k + 1) * 128],
                U_all[:, k * DY : (k + 1) * DY],
                ident,
            )
        nc.vector.tensor_copy(U_T, U_T_ps)

    # --- Ex2 = exp(hx+b1) stacked, Eu2 = exp(hu) stacked, cost ----------
    # Eu2 in bf16: read by both the ScalarE Ln1 head and the DVE chain (2-byte
    # operands keep the chain in DVE 2x perf mode).
    Ex2 = const.tile([128, NITER], F32)
    Eu2 = big.tile([128, M], BF16)
    cost = big.tile([128, M], F32)
    with tc.tile_pool(name="psumB", bufs=1, space="PSUM") as psB, tc.tile_pool(
        name="psumC", bufs=2, space="PSUM"
    ) as psC:
        # hx2[p, i] = (X @ W1a)[i + 64*(p>=64), p%64]; stacked columns.
        hx2_ps = psB.tile([128, NITER], F32)
        nc.tensor.matmul(hx2_ps[0:H, :], W1ab, X_T[:, 0:NITER], start=True, stop=True)
        nc.tensor.matmul(
            hx2_ps[H : 2 * H, :],
            W1ab,
            X_T[:, NITER : 2 * NITER],
            start=True,
            stop=True,
            tile_position=(0, 64),
        )
        nc.scalar.activation(Ex2, hx2_ps, AF.Exp, bias=b1s, scale=1.0)

        # hu2 = U @ W1b replicated on both partition halves via the doubled
        # stationary [W1b | W1b] (one matmul per chunk instead of two).
        hu2_ps = psB.tile([128, M], F32)
        for j in range(M // 512):
            sl = slice(j * 512, (j + 1) * 512)
            nc.tensor.matmul(hu2_ps[:, sl], W1bb2, U_T[:, sl], start=True, stop=True)
        nc.scalar.activation(Eu2, hu2_ps, AF.Exp, bias=0.0, scale=1.0)

        # cost = Y @ U.T  -> [128, 2048] f32
        for j in range(M // 512):
            sl = slice(j * 512, (j + 1) * 512)
            cost_ps = psC.tile([128, 512], F32, tag="cost")
            nc.tensor.matmul(cost_ps, Y_T, U_T[:, sl], start=True, stop=True)
            nc.vector.tensor_copy(cost[:, sl], cost_ps)

    # Kb2 = b2*128/ln2 + C: per-partition constant for the inverse-Mitchell
    # exp tail (bits = pre2*128/ln2 + Kb2).
    Kb2 = const.tile([128, 1], F32)
    nc.vector.tensor_scalar(
        out=Kb2, in0=b2s, scalar1=LOG2E128, scalar2=float(MITCH_C),
        op0=ALU.mult, op1=ALU.add,
    )

    # --- main loop -------------------------------------------------------
    # phi rows live in interleaved order: partition q holds row n(q)=q//2+64*(q%2)
    phi = big.tile([128, M], F32)
    h1_pool = ctx.enter_context(tc.tile_pool(name="h1", bufs=4))
    e2_pool = ctx.enter_context(tc.tile_pool(name="e2", bufs=4))
    h2_pool = ctx.enter_context(tc.tile_pool(name="h2", bufs=4))
    chain_pool = ctx.enter_context(tc.tile_pool(name="chain", bufs=2))
    stage_pool = ctx.enter_context(tc.tile_pool(name="stage", bufs=2))
    pre2_pool = ctx.enter_context(tc.tile_pool(name="pre2", bufs=1, space="PSUM"))
    phi_pool = ctx.enter_context(tc.tile_pool(name="phip", bufs=1, space="PSUM"))

    # Column layout per layer: [0, CS) ScalarE table-Ln | [CS, M-PW) DVE
    # Mitchell | [M-PW, M) Pool Mitchell.
    C2W = M - CSPLIT - POOLW  # layer-2 DVE-chain column count
    dsl = slice(CSPLIT, M - POOLW)
    psl = slice(M - POOLW, M)
    C1W = M - CSPLIT1 - POOLW1  # layer-1 DVE-chain column count
    dsl1 = slice(CSPLIT1, M - POOLW1)
    psl1 = slice(M - POOLW1, M)

    def emit_mitchell(eng, out_ap, t_bf16):
        # ln(t) ~ (bits_u16(t) - C) * ln2/128 for bf16 t >= 1 (one fused
        # tensor_scalar: int subtract, then multiply converts to float out).
        eng.tensor_scalar(
            out=out_ap, in0=t_bf16.bitcast(U16), scalar1=MITCH_C,
            scalar2=LN2 / 128.0, op0=ALU.subtract, op1=ALU.mult,
        )

    def emit_chain1(eng, out_ap, src_ap, W, i, tag):
        # layer-1 softplus tail: ln(Ex2[:,i]*src + 1) via Mitchell
        t1 = chain_pool.tile([128, W], BF16, tag=tag)
        eng.tensor_scalar(
            out=t1, in0=src_ap, scalar1=Ex2[:, i : i + 1],
            scalar2=1.0, op0=ALU.mult, op1=ALU.add,
        )
        emit_mitchell(eng, out_ap, t1)

    def emit_chain2(eng, out_ap, src_ap, W, tag):
        # layer-2 softplus tail: ln(src + 1) via Mitchell
        t2 = chain_pool.tile([128, W], BF16, tag=tag)
        eng.tensor_scalar_add(t2, src_ap, 1.0)
        emit_mitchell(eng, out_ap, t2)

    def emit_ln1(i):
        # softplus1: h1 = Ln(Ex2[:,i] * Eu2 + 1)   (rows i and i+64)
        h1 = h1_pool.tile([128, M], BF16, tag="h1", name=f"h1_{i}")
        if CSPLIT1:
            nc.scalar.activation(
                h1[:, 0:CSPLIT1], Eu2[:, 0:CSPLIT1], AF.Ln, bias=1.0,
                scale=Ex2[:, i : i + 1],
            )
        if C1W:
            emit_chain1(nc.vector, h1[:, dsl1], Eu2[:, dsl1], C1W, i, "t1d")
        if POOLW1:
            emit_chain1(nc.gpsimd, h1[:, psl1], Eu2[:, psl1], POOLW1, i, "t1p")
        return h1

    esl = slice(M - EXPW, M)
    assert EXPW == 0 or EXPW == POOLW, "exp tail must equal the Pool range"

    def emit_mm1(i, h1):
        # pre2 = W2s.T @ h1  (block-diag -> both halves independently)
        pre2_ps = pre2_pool.tile([128, M], F32, tag="pre2", name=f"pre2_{i}")
        for j in range(M // 512):
            sl = slice(j * 512, (j + 1) * 512)
            nc.tensor.matmul(pre2_ps[:, sl], W2s, h1[:, sl], start=True, stop=True)
        e2 = e2_pool.tile([128, M - EXPW], BF16, tag="e2", name=f"e2_{i}")
        et = (
            e2_pool.tile([128, EXPW], BF16, tag="et", name=f"et_{i}")
            if EXPW
            else None
        )
        return pre2_ps, e2, et

    def emit_exp_tail(pre2_ps, e2, et):
        # inverse-Mitchell exp for the tail columns into its OWN tile (a
        # second writer on e2 would add semaphore waits to every downstream
        # ScalarE op). Emitted AFTER the other DVE work of the body so it
        # never head-of-line-blocks the chains while waiting on mm1.
        if EXPW:
            nc.vector.tensor_scalar(
                out=et.bitcast(U16), in0=pre2_ps[:, esl],
                scalar1=LOG2E128, scalar2=Kb2, op0=ALU.mult, op1=ALU.add,
            )

    # Software pipeline: keep ScalarE's FIFO fed — emit Ln1 two iterations
    # ahead and mm1 one iteration ahead, so PE work overlaps the Ln1/chain
    # window instead of stalling the next Exp.
    h1_ahead = {0: emit_ln1(0), 1: emit_ln1(1)}
    pre2_ahead = {0: emit_mm1(0, h1_ahead.pop(0))}
    emit_exp_tail(*pre2_ahead[0])
    e2_live = {}

    def emit_h2_phi(k, phi_quad):
        # finish softplus2 for iteration k from its (already long-done) e2,
        # then its phi matmul quad piece. Runs one iteration behind Exp2 so
        # the DVE/Pool chains never wait on a just-issued ScalarE op.
        e2, et = e2_live.pop(k)
        h2 = h2_pool.tile([128, M], BF16, tag="h2")
        if CSPLIT:
            nc.scalar.activation(
                h2[:, 0:CSPLIT], e2[:, 0:CSPLIT], AF.Ln, bias=1.0, scale=1.0
            )
        if C2W:
            emit_chain2(nc.vector, h2[:, dsl], e2[:, dsl], C2W, "t2d")
        if POOLW:
            psrc = et if EXPW else e2[:, psl]
            emit_chain2(nc.gpsimd, h2[:, psl], psrc, POOLW, "t2p")
        q = k % 4
        for j in range(M // 512):
            sl = slice(j * 512, (j + 1) * 512)
            nc.tensor.matmul(
                phi_quad[32 * q : 32 * q + 32, sl], W3s, h2[:, sl],
                start=True, stop=True,
                tile_position=(0, 32 * q) if q else None,
            )
        if q == 3:
            # PSUM is not a legal DMA source: stage through SBUF on the DVE.
            phi_stage = stage_pool.tile([128, M], F32, tag="stage")
            nc.vector.tensor_copy(phi_stage, phi_quad)
            g = k // 4
            for qq in range(4):
                r = 2 * (4 * g + qq)
                nc.sync.dma_start(
                    out=phi[r : r + 2, :],
                    in_=phi_stage[32 * qq : 32 * qq + 2, :],
                )

    # phi quad tiles are consumed one iteration late, so allocate per quad
    # of the DELAYED index k = i-1.
    phi_quads = {}
    for i in range(NITER):
        pre2_ps, e2, et = pre2_ahead.pop(i)

        # softplus2 part 1: e2 = Exp(pre2 + b2) on ScalarE, table-Exp on
        # [0, M-EXPW) (the tail was already computed by the DVE into et).
        nc.scalar.activation(e2, pre2_ps[:, 0 : M - EXPW], AF.Exp,
                             bias=b2s, scale=1.0)
        e2_live[i] = (e2, et)

        # hoist next iteration's mm1 so PE runs it during the Ln1/chain window
        if i + 1 < NITER:
            pre2_ahead[i + 1] = emit_mm1(i + 1, h1_ahead.pop(i + 1))

        # softplus2 part 2 + phi for the PREVIOUS iteration
        k = i - 1
        if k >= 0:
            if k % 4 == 0:
                phi_quads[k // 4] = phi_pool.tile([128, M], F32, tag="phi", name=f"phiq_{k}")
            emit_h2_phi(k, phi_quads[k // 4])

        # exp tail for iteration i+1 (behind the chains on the DVE queue)
        if i + 1 < NITER:
            emit_exp_tail(*pre2_ahead[i + 1])

        if i + 2 < NITER:
            h1_ahead[i + 2] = emit_ln1(i + 2)

    # drain the last delayed iteration
    k = NITER - 1
    if k % 4 == 0:
        phi_quads[k // 4] = phi_pool.tile([128, M], F32, tag="phi", name=f"phiq_{k}")
    emit_h2_phi(k, phi_quads[k // 4])

    # --- final: psi = rowmax(cost - phi) - EPS*log(M) - b3 ---------------
    # With EPS=1e-7 the f32 logsumexp collapses to the row max: the exp of
    # the second-best gap underflows, so the correction is exactly -EPS*log(M)
    # (bounded by EPS*log(M) ~ 7.6e-7 in all cases — far below tolerance).
    fin = ctx.enter_context(tc.tile_pool(name="fin", bufs=1))
    # NOTE: vector.tensor_tensor_reduce wedges the device (NRT_EXEC_UNIT_
    # UNRECOVERABLE) on this stack — use separate sub + reduce_max.
    slack = big.tile([128, M], F32)
    rowmax = fin.tile([128, 1], F32)
    nc.vector.tensor_sub(slack, cost, phi)
    nc.vector.reduce_max(out=rowmax, in_=slack, axis=mybir.AxisListType.X)
    base = fin.tile([128, 1], F32)
    # base = -b3 - EPS*log(M)
    nc.vector.tensor_scalar(
        out=base, in0=b3s, scalar1=-1.0, scalar2=-EPS * math.log(M),
        op0=ALU.mult, op1=ALU.add,
    )
    psi = fin.tile([128, 1], F32)
    nc.vector.tensor_add(psi, rowmax, base)
    # psi partition q holds row n(q)=q//2+64*(q%2); un-permute via the DRAM AP.
    out_perm = bass.AP(tensor=out.tensor, offset=out.offset, ap=[[1, NITER], [NITER, 2]])
    nc.sync.dma_start(out=out_perm, in_=psi)


def kernel(**inputs):
    if "nc" not in _CACHE:
        _CACHE["nc"] = build_program()
    nc = _CACHE["nc"]

    f32 = lambda a: np.ascontiguousarray(np.asarray(a, dtype=np.float32))
    X = f32(inputs["X"])
    U = f32(inputs["U"])
    Y = f32(inputs["Y"])
    shared = dict(
        U=U,
        W1=f32(inputs["W1"]),
        b1=f32(inputs["b1"]),
        W2=f32(inputs["W2"]),
        b2=f32(inputs["b2"]),
        W3=f32(inputs["W3"]),
        b3=f32(inputs["b3"]),
    )
    in_maps = [
        dict(
            X=X[c * NLOC : (c + 1) * NLOC],
            Y=Y[c * NLOC : (c + 1) * NLOC],
            **shared,
        )
        for c in range(N_CORES)
    ]
    res = run_bass_kernel_spmd(nc, in_maps, core_ids=list(range(N_CORES)))
    return np.concatenate([res.results[c]["out"] for c in range(N_CORES)], axis=0)


if __name__ == "__main__":
    rng = np.random.default_rng(0)
    ins = {
        "X": rng.standard_normal((N, DX), dtype=np.float32),
        "U": rng.standard_normal((M, DY), dtype=np.float32),
        "Y": rng.standard_normal((N, DY), dtype=np.float32),
        "W1": (rng.standard_normal((DX + DY, H)) * 0.1).astype(np.float32),
        "b1": np.zeros(H, np.float32),
        "W2": (rng.standard_normal((H, H)) * 0.1).astype(np.float32),
        "b2": np.zeros(H, np.float32),
        "W3": (rng.standard_normal((H, 1)) * 0.1).astype(np.float32),
        "b3": np.zeros(1, np.float32),
    }
    out = kernel(**ins)
    print(out.shape, out[:4, 0])



# revision 3
# speedup vs baseline: 10.2402x; 2.9069x over previous
"""Trainium2 Bass kernel for nn_EntropicOTQuantileRegression.

Reference computation (N=1024, M=2048, DX=48, DY=8, H=64, EPS=1e-7):
    hx = X @ W1[:DX]                                  [n, h]
    hu = U @ W1[DX:]                                  [m, h]
    h1 = softplus(hx[:,None,:] + hu[None,:,:] + b1)   [n, m, h]
    h2 = softplus(h1 @ W2 + b2)                       [n, m, h]
    phi = (h2 @ W3)[..., 0] + b3[0]                   [n, m]
    slack = Y @ U.T - phi
    psi = EPS*(logsumexp((slack - rowmax)/EPS, axis=1) - log(M)) + rowmax

Sharding: data-parallel over n. Each of the 8 cores gets 128 rows of X/Y and
replicates U + MLP weights. No collectives.

Design (HW-measured 1341us baseline -> ~250us -> this version):
- Partition layout stacks two n-rows (h=64: 128 partitions hold rows i, i+64).
- Layer-1 pre-activation is separable: exp(hx+hu+b1) = exp(hx+b1)*exp(hu), so
  softplus1 = ln(Ex2[:,i]*Eu2 + 1), computed entirely on the DVE as a 2-op
  bf16 "Mitchell" chain (t1 = Ex2*Eu2+1; h1 = (bits(t1)-C)*ln2/128).
- Layer-2 uses softplus(x) = -ln(sigmoid(-x)): ONE ScalarE sigmoid op per
  pre2 half (s2 = sigmoid(-(pre2+b2)), table-exact, bf16 out) plus ONE DVE
  Mitchell op (h2 = (C-bits(s2))*ln2/128 >= 0). This removes the old
  Exp+Ln pair (two full ScalarE passes + chains) from the loop; the loop
  touches only the sigmoid act table -> no table thrash.
- The Pool/GPSIMD engine is NOT used in the loop: HW A/B showed dependent
  Pool ops cost ~1.1ms in wake-up stalls (the old 1.34ms baseline collapsed
  to ~250us just by moving Pool work to the DVE).
- pre2 lives in PSUM as TWO half-width tiles ([128,1024] f32, bufs=2, 4
  banks total) so mm1(i+1, h) only waits on sigmoid(i, h) - PE and ScalarE
  ping-pong halves instead of serializing on one full-width buffer.
- phi is accumulated quad-packed in PSUM (4 iterations per [128, M] tile via
  tile_position column offsets), staged to SBUF once per quad (DVE, with an
  optional ScalarE Copy share - sigmoid table contains Copy, no reload).
- Epilogue: EPS=1e-7 collapses the f32 logsumexp to a row max exactly, so
  psi = rowmax(cost - phi) - EPS*log(M) - b3 with no Exp/Ln pass.
- Prologue: constants ride the ACT hardware DMA queue while X/Y/U stream on
  the SP queue (U in one strided DMA); big prologue matmuls are bf16 with
  doubled stationaries to dodge the 4x f32 cost at the cold-PE low pstate.
"""

import math
from contextlib import ExitStack

import numpy as np

import concourse.bass as bass
import concourse.bacc as bacc
import concourse.tile as tile
from concourse import mybir
from concourse.bass_utils import run_bass_kernel_spmd
from concourse.masks import make_identity

# Problem constants (hardcoded; kernel.py must be self-contained).
N, M = 1024, 2048
DX, DY = 48, 8
H = 64
EPS = 1e-7
N_CORES = 8
NLOC = N // N_CORES  # 128 rows per core
F32 = mybir.dt.float32
BF16 = mybir.dt.bfloat16
U16 = mybir.dt.uint16
AF = mybir.ActivationFunctionType
ALU = mybir.AluOpType

# Mitchell bit-trick: for bf16 t > 0, ln(t) ~ (bits_u16(t) - C) * ln2/128,
# since bits(t) = 128*(log2 t + 127 + eps(u)), eps in [0, 0.0861]. C centers
# eps; worst-case h-error ~0.03 which is ~15x under the psi error budget
# (numpy end-to-end: l2 rel 1.5e-3).
LN2 = math.log(2.0)
MITCH_C = 16256 - 6  # 127<<7 minus eps-centering

# Stage split: per quad, the last STAGESC columns of the PSUM->SBUF phi copy
# run on ScalarE (AF.Copy, same act table as Sigmoid), the rest on the DVE.
STAGESC = 0

_CACHE = {}


def _patch_act_tables():
    """Make Exp/Ln resolve uniquely to the combined natural_log_exp_and_others
    table so `insert_act_table_loads` hoists ONE load instead of thrashing.

    (Prologue uses Exp; the main loop uses only Sigmoid/Copy, which share the
    sigmoid_and_others table -> 2 table loads total.)
    """
    if getattr(bacc, "_act_tables_patched", False):
        return
    orig = bacc.get_activation_tables
    AFT = mybir.ActivationFunctionType

    def patched(arch):
        tabs = dict(orig(arch))
        combined = "natural_log_exp_and_others"
        if combined in tabs and {AFT.Exp, AFT.Ln} <= tabs[combined]:
            tabs = {
                name: (s if name == combined else s - {AFT.Exp, AFT.Ln})
                for name, s in tabs.items()
            }
        return tabs

    bacc.get_activation_tables = patched
    bacc._act_tables_patched = True


def build_program(repeats=1, loop_n=0, stagesc=None, **_ignored):
    global STAGESC
    if stagesc is not None:
        STAGESC = stagesc
    _patch_act_tables()
    nc = bacc.Bacc(
        "TRN2",
        target_bir_lowering=False,
        debug=False,
        enable_asserts=False,
        num_devices=N_CORES,
    )

    X = nc.dram_tensor("X", (NLOC, DX), F32, kind="ExternalInput").ap()
    U = nc.dram_tensor("U", (M, DY), F32, kind="ExternalInput").ap()
    Y = nc.dram_tensor("Y", (NLOC, DY), F32, kind="ExternalInput").ap()
    W1 = nc.dram_tensor("W1", (DX + DY, H), F32, kind="ExternalInput").ap()
    b1 = nc.dram_tensor("b1", (H,), F32, kind="ExternalInput").ap()
    W2 = nc.dram_tensor("W2", (H, H), F32, kind="ExternalInput").ap()
    b2 = nc.dram_tensor("b2", (H,), F32, kind="ExternalInput").ap()
    W3 = nc.dram_tensor("W3", (H, 1), F32, kind="ExternalInput").ap()
    b3 = nc.dram_tensor("b3", (1,), F32, kind="ExternalInput").ap()
    out = nc.dram_tensor("out", (NLOC, 1), F32, kind="ExternalOutput").ap()

    with tile.TileContext(nc) as tc:
        if loop_n:
            with tc.For_i(0, loop_n, 1):
                with ExitStack() as ctx:
                    _body(ctx, tc, nc, X, U, Y, W1, b1, W2, b2, W3, b3, out)
        else:
            for _ in range(repeats):
                with ExitStack() as ctx:
                    _body(ctx, tc, nc, X, U, Y, W1, b1, W2, b2, W3, b3, out)

    nc.compile()
    return nc


def _body(ctx, tc, nc, X, U, Y, W1, b1, W2, b2, W3, b3, out):
    NITER = NLOC // 2  # 64: each iteration handles rows (i, i+64)
    HALF = M // 2  # 1024: pre2 PSUM half width (2 banks each)

    const = ctx.enter_context(tc.tile_pool(name="const", bufs=1))
    big = ctx.enter_context(tc.tile_pool(name="big", bufs=1))

    # --- small SBUF constants -------------------------------------------
    # Each dma_start costs ~625ns of queue time, so constants are fused into
    # single DMAs (stride-0 repeat APs for the x2 partition stacking) and
    # routed via the ScalarE hardware DMA queue so X/Y/U can stream on the SP
    # queue in parallel. (gpsimd dma_start is SWDGE: ~1us serial on Pool.)
    ident = const.tile([128, 128], F32)
    make_identity(nc, ident)

    rep2 = lambda t, inner: bass.AP(tensor=t, offset=0, ap=[[0, 2]] + inner)

    W1a = const.tile([DX, H], F32)
    nc.scalar.dma_start(out=W1a, in_=W1[0:DX, :])
    W1b = const.tile([DY, H], F32)
    nc.scalar.dma_start(out=W1b, in_=W1[DX : DX + DY, :])

    # b1/b2 stacked twice on 128 partitions: partition p holds b[p % 64]
    b1s = const.tile([128, 1], F32)
    nc.scalar.dma_start(out=b1s, in_=rep2(b1.tensor, [[1, H]]))
    b2s = const.tile([128, 1], F32)
    nc.scalar.dma_start(out=b2s, in_=rep2(b2.tensor, [[1, H]]))
    b3s = const.tile([128, 1], F32)
    nc.scalar.dma_start(out=b3s, in_=b3.unsqueeze(1).partition_broadcast(128))

    # nb2s = -b2 (bias for the sigmoid: s2 = sigmoid(-pre2 - b2))
    nb2s = const.tile([128, 1], F32)
    nc.vector.tensor_scalar(
        out=nb2s, in0=b2s, scalar1=-1.0, scalar2=0.0, op0=ALU.mult, op1=ALU.add
    )

    # W2 block-diagonal stack [128,128] bf16: [[W2, 0], [0, W2]]
    W2f = const.tile([128, H], F32)
    nc.scalar.dma_start(out=W2f, in_=rep2(W2.tensor, [[H, H], [1, H]]))
    W2s = const.tile([128, 128], BF16)
    nc.vector.memset(W2s, 0.0)
    nc.vector.tensor_copy(W2s[0:H, 0:H], W2f[0:H, :])
    nc.vector.tensor_copy(W2s[H : 2 * H, H : 2 * H], W2f[H : 2 * H, :])

    # W3 stack [128, 32] bf16: cols 0/1 = the two W3 halves, cols 2..31 zero
    # (32-wide so each quad phi matmul writes a full 32-partition col group).
    W3f = const.tile([128, 1], F32)
    nc.scalar.dma_start(out=W3f, in_=rep2(W3.tensor, [[1, H]]))
    W3s = const.tile([128, 32], BF16)
    nc.vector.memset(W3s, 0.0)
    nc.vector.tensor_copy(W3s[0:H, 0:1], W3f[0:H, :])
    nc.vector.tensor_copy(W3s[H : 2 * H, 1:2], W3f[H : 2 * H, :])

    # bf16 doubled stationaries so the big prologue matmuls run 1 col/cycle
    # (f32 matmuls cost 4x cycles and hit the cold-PE low pstate).
    W1ab = const.tile([DX, H], BF16)
    nc.vector.tensor_copy(W1ab, W1a)
    W1bb2 = const.tile([DY, 128], BF16)  # [W1b | W1b]
    nc.vector.tensor_copy(W1bb2[:, 0:H], W1b)
    nc.vector.tensor_copy(W1bb2[:, H : 2 * H], W1b)

    # --- transposes (PE) -------------------------------------------------
    X_T = const.tile([DX, 128], BF16)  # X^T
    Y_T = const.tile([DY, 128], BF16)  # Y^T
    U_T = const.tile([DY, M], BF16)  # U^T
    with tc.tile_pool(name="psumA", bufs=1, space="PSUM") as psA, tc.tile_pool(
        name="ld", bufs=4
    ) as ld:
        X_sb = ld.tile([128, DX], F32, tag="xy")
        nc.sync.dma_start(out=X_sb, in_=X)
        X_T_ps = psA.tile([DX, 128], F32, tag="xyt")
        nc.tensor.transpose(X_T_ps, X_sb, ident)
        nc.vector.tensor_copy(X_T, X_T_ps)

        # Y rows loaded in interleaved order q = 2i+p <-> n = i + 64p, so that
        # cost rows line up with the phi layout written by the main loop.
        Y_sb = ld.tile([128, DY], F32, tag="xy")
        Y_perm = bass.AP(
            tensor=Y.tensor,
            offset=Y.offset,
            ap=[[DY, NITER], [NITER * DY, 2], [1, DY]],
        )
        nc.sync.dma_start(out=Y_sb, in_=Y_perm)
        Y_T_ps = psA.tile([DY, 128], F32, tag="xyt")
        nc.tensor.transpose(Y_T_ps, Y_sb, ident)
        nc.vector.tensor_copy(Y_T, Y_T_ps)

        # U in ONE DMA: partition p, column group k holds U[k*128+p, :]
        U_all = ld.tile([128, (M // 128) * DY], F32, tag="uall")
        U_perm = bass.AP(
            tensor=U.tensor,
            offset=0,
            ap=[[DY, 128], [128 * DY, M // 128], [1, DY]],
        )
        nc.sync.dma_start(out=U_all, in_=U_perm)
        U_T_ps = psA.tile([DY, M], F32)
        for k in range(M // 128):
            nc.tensor.transpose(
                U_T_ps[:, k * 128 : (k + 1) * 128],
                U_all[:, k * DY : (k + 1) * DY],
                ident,
            )
        nc.vector.tensor_copy(U_T, U_T_ps)

    # --- Ex2 = exp(hx+b1) stacked, Eu2 = exp(hu) stacked, cost ----------
    Ex2 = const.tile([128, NITER], F32)
    Eu2 = big.tile([128, M], BF16)
    cost = big.tile([128, M], F32)
    with tc.tile_pool(name="psumB", bufs=1, space="PSUM") as psB, tc.tile_pool(
        name="psumC", bufs=2, space="PSUM"
    ) as psC:
        # hx2[p, i] = (X @ W1a)[i + 64*(p>=64), p%64]; stacked columns.
        hx2_ps = psB.tile([128, NITER], F32)
        nc.tensor.matmul(hx2_ps[0:H, :], W1ab, X_T[:, 0:NITER], start=True, stop=True)
        nc.tensor.matmul(
            hx2_ps[H : 2 * H, :],
            W1ab,
            X_T[:, NITER : 2 * NITER],
            start=True,
            stop=True,
            tile_position=(0, 64),
        )
        nc.scalar.activation(Ex2, hx2_ps, AF.Exp, bias=b1s, scale=1.0)

        # hu2 = U @ W1b replicated on both partition halves via the doubled
        # stationary [W1b | W1b] (one matmul per chunk instead of two).
        hu2_ps = psB.tile([128, M], F32)
        for j in range(M // 512):
            sl = slice(j * 512, (j + 1) * 512)
            nc.tensor.matmul(hu2_ps[:, sl], W1bb2, U_T[:, sl], start=True, stop=True)
        nc.scalar.activation(Eu2, hu2_ps, AF.Exp, bias=0.0, scale=1.0)

        # cost = Y @ U.T  -> [128, 2048] f32
        for j in range(M // 512):
            sl = slice(j * 512, (j + 1) * 512)
            cost_ps = psC.tile([128, 512], F32, tag="cost")
            nc.tensor.matmul(cost_ps, Y_T, U_T[:, sl], start=True, stop=True)
            nc.vector.tensor_copy(cost[:, sl], cost_ps)

    # --- main loop -------------------------------------------------------
    # phi rows live in interleaved order: partition q holds row n(q)=q//2+64*(q%2)
    phi = big.tile([128, M], F32)
    h1_pool = ctx.enter_context(tc.tile_pool(name="h1", bufs=4))
    t1_pool = ctx.enter_context(tc.tile_pool(name="t1", bufs=2))
    s2_pool = ctx.enter_context(tc.tile_pool(name="s2", bufs=6))
    h2_pool = ctx.enter_context(tc.tile_pool(name="h2", bufs=2))
    stage_pool = ctx.enter_context(tc.tile_pool(name="stage", bufs=2))
    pre2_pool = ctx.enter_context(tc.tile_pool(name="pre2", bufs=2, space="PSUM"))
    phi_pool = ctx.enter_context(tc.tile_pool(name="phip", bufs=1, space="PSUM"))

    def emit_ln1(i):
        # softplus1 all-DVE: t1 = Ex2[:,i]*Eu2 + 1 (bf16), h1 = Mitchell(t1)
        h1 = h1_pool.tile([128, M], BF16, tag="h1", name=f"h1_{i}")
        t1 = t1_pool.tile([128, M], BF16, tag="t1", name=f"t1_{i}")
        nc.vector.tensor_scalar(
            out=t1, in0=Eu2, scalar1=Ex2[:, i : i + 1], scalar2=1.0,
            op0=ALU.mult, op1=ALU.add,
        )
        nc.vector.tensor_scalar(
            out=h1, in0=t1.bitcast(U16), scalar1=MITCH_C, scalar2=LN2 / 128.0,
            op0=ALU.subtract, op1=ALU.mult,
        )
        return h1

    def emit_mm1(i, h1):
        # pre2 = W2s.T @ h1 per half (block-diag -> both row-halves at once)
        halves = []
        for h in range(2):
            pre2_ps = pre2_pool.tile(
                [128, HALF], F32, tag="pre2", name=f"pre2_{i}_{h}"
            )
            for j in range(2):
                sl_o = slice(j * 512, (j + 1) * 512)
                sl_i = slice(h * HALF + j * 512, h * HALF + (j + 1) * 512)
                nc.tensor.matmul(
                    pre2_ps[:, sl_o], W2s, h1[:, sl_i], start=True, stop=True
                )
            halves.append(pre2_ps)
        return halves

    def emit_sig(i, pre2_halves):
        # softplus2 part 1: s2 = sigmoid(-(pre2 + b2)) on ScalarE (bf16 out)
        s2h = []
        for h in range(2):
            s2 = s2_pool.tile([128, HALF], BF16, tag="s2", name=f"s2_{i}_{h}")
            nc.scalar.activation(
                s2, pre2_halves[h], AF.Sigmoid, bias=nb2s, scale=-1.0
            )
            s2h.append(s2)
        return s2h

    def emit_h2_phi(k, s2h, phi_quad):
        # softplus2 part 2: h2 = -ln(s2) via one DVE Mitchell op per half,
        # then the phi matmul quad piece for iteration k.
        h2 = h2_pool.tile([128, M], BF16, tag="h2", name=f"h2_{k}")
        for h in range(2):
            nc.vector.tensor_scalar(
                out=h2[:, h * HALF : (h + 1) * HALF], in0=s2h[h].bitcast(U16),
                scalar1=MITCH_C, scalar2=-LN2 / 128.0,
                op0=ALU.subtract, op1=ALU.mult,
            )
        q = k % 4
        for j in range(M // 512):
            sl = slice(j * 512, (j + 1) * 512)
            nc.tensor.matmul(
                phi_quad[32 * q : 32 * q + 32, sl], W3s, h2[:, sl],
                start=True, stop=True,
                tile_position=(0, 32 * q) if q else None,
            )
        if q == 3:
            # PSUM is not a legal DMA source: stage through SBUF (DVE, with
            # an optional ScalarE AF.Copy share - same act table as Sigmoid).
            phi_stage = stage_pool.tile([128, M], F32, tag="stage")
            if STAGESC:
                nc.vector.tensor_copy(
                    phi_stage[:, 0 : M - STAGESC], phi_quad[:, 0 : M - STAGESC]
                )
                nc.scalar.activation(
                    phi_stage[:, M - STAGESC : M], phi_quad[:, M - STAGESC : M],
                    AF.Copy, bias=0.0, scale=1.0,
                )
            else:
                nc.vector.tensor_copy(phi_stage, phi_quad)
            g = k // 4
            for qq in range(4):
                r = 2 * (4 * g + qq)
                nc.sync.dma_start(
                    out=phi[r : r + 2, :],
                    in_=phi_stage[32 * qq : 32 * qq + 2, :],
                )

    # Software pipeline: Ln1 two iterations ahead (DVE), mm1 one ahead (PE),
    # sigmoid at i (ScalarE), h2+phi one behind (DVE+PE). pre2 half-buffers
    # let mm1(i+1, h) start as soon as sigmoid(i, h) drains its half.
    h1_ahead = {0: emit_ln1(0), 1: emit_ln1(1)}
    pre2_ahead = {0: emit_mm1(0, h1_ahead.pop(0))}
    s2_live = {}
    phi_quads = {}

    for i in range(NITER):
        s2_live[i] = emit_sig(i, pre2_ahead.pop(i))

        # hoist next iteration's mm1 so PE runs it during the sigmoid window
        if i + 1 < NITER:
            pre2_ahead[i + 1] = emit_mm1(i + 1, h1_ahead.pop(i + 1))

        # softplus2 part 2 + phi for the PREVIOUS iteration
        k = i - 1
        if k >= 0:
            if k % 4 == 0:
                phi_quads[k // 4] = phi_pool.tile(
                    [128, M], F32, tag="phi", name=f"phiq_{k}"
                )
            emit_h2_phi(k, s2_live.pop(k), phi_quads[k // 4])

        if i + 2 < NITER:
            h1_ahead[i + 2] = emit_ln1(i + 2)

    # drain the last delayed iteration
    k = NITER - 1
    if k % 4 == 0:
        phi_quads[k // 4] = phi_pool.tile([128, M], F32, tag="phi", name=f"phiq_{k}")
    emit_h2_phi(k, s2_live.pop(k), phi_quads[k // 4])

    # --- final: psi = rowmax(cost - phi) - EPS*log(M) - b3 ---------------
    # With EPS=1e-7 the f32 logsumexp collapses to the row max: the exp of
    # the second-best gap underflows, so the correction is exactly -EPS*log(M)
    # (bounded by EPS*log(M) ~ 7.6e-7 in all cases - far below tolerance).
    fin = ctx.enter_context(tc.tile_pool(name="fin", bufs=1))
    # NOTE: vector.tensor_tensor_reduce wedges the device (NRT_EXEC_UNIT_
    # UNRECOVERABLE) on this stack - use separate sub + reduce_max.
    slack = big.tile([128, M], F32)
    rowmax = fin.tile([128, 1], F32)
    nc.vector.tensor_sub(slack, cost, phi)
    nc.vector.reduce_max(out=rowmax, in_=slack, axis=mybir.AxisListType.X)
    base = fin.tile([128, 1], F32)
    # base = -b3 - EPS*log(M)
    nc.vector.tensor_scalar(
        out=base, in0=b3s, scalar1=-1.0, scalar2=-EPS * math.log(M),
        op0=ALU.mult, op1=ALU.add,
    )
    psi = fin.tile([128, 1], F32)
    nc.vector.tensor_add(psi, rowmax, base)
    # psi partition q holds row n(q)=q//2+64*(q%2); un-permute via the DRAM AP.
    out_perm = bass.AP(tensor=out.tensor, offset=out.offset, ap=[[1, NITER], [NITER, 2]])
    nc.sync.dma_start(out=out_perm, in_=psi)


def kernel(**inputs):
    if "nc" not in _CACHE:
        _CACHE["nc"] = build_program()
    nc = _CACHE["nc"]

    f32 = lambda a: np.ascontiguousarray(np.asarray(a, dtype=np.float32))
    X = f32(inputs["X"])
    U = f32(inputs["U"])
    Y = f32(inputs["Y"])
    shared = dict(
        U=U,
        W1=f32(inputs["W1"]),
        b1=f32(inputs["b1"]),
        W2=f32(inputs["W2"]),
        b2=f32(inputs["b2"]),
        W3=f32(inputs["W3"]),
        b3=f32(inputs["b3"]),
    )
    in_maps = [
        dict(
            X=X[c * NLOC : (c + 1) * NLOC],
            Y=Y[c * NLOC : (c + 1) * NLOC],
            **shared,
        )
        for c in range(N_CORES)
    ]
    res = run_bass_kernel_spmd(nc, in_maps, core_ids=list(range(N_CORES)))
    return np.concatenate([res.results[c]["out"] for c in range(N_CORES)], axis=0)


if __name__ == "__main__":
    rng = np.random.default_rng(0)
    ins = {
        "X": rng.standard_normal((N, DX), dtype=np.float32),
        "U": rng.standard_normal((M, DY), dtype=np.float32),
        "Y": rng.standard_normal((N, DY), dtype=np.float32),
        "W1": (rng.standard_normal((DX + DY, H)) * 0.1).astype(np.float32),
        "b1": np.zeros(H, np.float32),
        "W2": (rng.standard_normal((H, H)) * 0.1).astype(np.float32),
        "b2": np.zeros(H, np.float32),
        "W3": (rng.standard_normal((H, 1)) * 0.1).astype(np.float32),
        "b3": np.zeros(1, np.float32),
    }
    out = kernel(**ins)
    print(out.shape, out[:4, 0])
